# revision 10
# baseline (speedup 1.0000x reference)
"""Causal (cumulative) LayerNorm Trainium2 Bass kernel.

Full-input contract: kernel(inputs, gamma, beta) takes the full
(B=8, K=16000, H=256) f32 tensor, shards batch across 8 NeuronCores
(one sample per core), and returns the full (8, 16000, 256) output.

Per-core algorithm (x is (K, H)):
  rowsum[k]   = sum_h x[k, h]
  rowsumsq[k] = sum_h x[k, h]^2
  csum = cumsum(rowsum); cpow = cumsum(rowsumsq)
  mean[k] = csum[k] / (H*(k+1));  msq[k] = cpow[k] / (H*(k+1))
  var[k] = msq[k] - mean[k]^2
  out[k, h] = gamma[h] * (x[k, h] - mean[k]) / sqrt(var[k] + EPS) + beta[h]

Layout: K = 16000 = 125 row-tiles x 128 rows, SBUF-resident tile-major
as (128 part, 125 tile, 256 h), row k = t*128 + p at [p, t, :].
Per-row sums come from one bn_stats per tile (even/odd mean/M2). The
even/odd merge and the tile-major -> chunk-major transpose are fused
into accumulated PE matmuls against an identity (PE fp32 is
full-precision; verified ~6e-8 rel): ps_s = me.T@I + mo.T@I,
ps_p = (128*me^2).T@I + (128*mo^2).T@I + m2e.T@I + m2o.T@I.

Tiles are processed in variable-size segments (small first segment so
the output pass starts early, small last segment for a short drain).
Per segment: chunk-major (ts, 128) scans along the free axis (fp32
vector scan), cross-chunk carry via a (2,)-wide transpose pair seeded
by a running inter-segment carry cell, per-row scale/bias (rstd,
-mean*rstd) transposed back tile-major. Output pass per group of 5
tiles: per-tile affine (scalar engine; a couple of groups on gpsimd),
batched gamma multiply (gpsimd mostly, some vector), store.

Loads ride the SP HWDGE queue and stores the Activation HWDGE queue so
a store waiting on compute never blocks later loads. DMA is the
roofline: ~32.8 MB @ ~358 GB/s/core.
"""

import numpy as np

import concourse.bass as bass
import concourse.bacc as bacc
import concourse.tile as tile
from concourse import mybir
from concourse.bass_utils import run_bass_kernel_spmd

EPS = 1e-8
B, K, H = 8, 16000, 256
P = 128                 # SBUF partitions
NT = K // P             # 125 row-tiles per sample
G = 5                   # tiles per DMA group / output group
NG = NT // G            # 25 groups
SEGS = [10, 25, 30, 30, 20, 10]      # tiles per scan segment (sum = NT)
assert sum(SEGS) == NT and all(ts % G == 0 for ts in SEGS)
F32 = mybir.dt.float32
ALU = mybir.AluOpType
ACTF = mybir.ActivationFunctionType

GAMMA_DVE_GROUPS = {0, 5, 10, 15, 20}   # gamma-mult on vector, rest gpsimd
AFFINE_GPS_GROUPS = {7, 15}             # affine on gpsimd, rest scalar


def _build(use_beta: bool):
    nc = bacc.Bacc("TRN2", target_bir_lowering=False, debug=False)

    x = nc.declare_dram_parameter("x", [K, H], F32, isOutput=False)
    gamma_b = nc.declare_dram_parameter("gamma_b", [P, H], F32, isOutput=False)
    beta_b = (
        nc.declare_dram_parameter("beta_b", [P, H], F32, isOutput=False)
        if use_beta
        else None
    )
    ident = nc.declare_dram_parameter("ident", [P, P], F32, isOutput=False)
    invc_m = nc.declare_dram_parameter("invc_m", [NT, P], F32, isOutput=False)
    invc_p = nc.declare_dram_parameter("invc_p", [NT, P], F32, isOutput=False)
    y = nc.declare_dram_parameter("y", [K, H], F32, isOutput=True)

    xr = x.rearrange("(t p) h -> p t h", p=P)   # [128, 125, 256]
    yr = y.rearrange("(t p) h -> p t h", p=P)

    with tile.TileContext(nc) as tc:
        with (
            tc.tile_pool(name="singles", bufs=1) as singles,
            tc.tile_pool(name="xpool", bufs=NG) as xpool,
            tc.tile_pool(name="opool", bufs=8) as opool,
            tc.tile_pool(name="segp", bufs=2) as segp,
            tc.tile_pool(name="psum", bufs=1, space="PSUM") as psum,
        ):
            sb_gamma = singles.tile([P, H], F32)
            nc.sync.dma_start(out=sb_gamma[:], in_=gamma_b[:])
            if use_beta:
                sb_beta = singles.tile([P, H], F32)
                nc.sync.dma_start(out=sb_beta[:], in_=beta_b[:])
            sb_ident = singles.tile([P, P], F32)
            nc.sync.dma_start(out=sb_ident[:], in_=ident[:])

            sb_invm = []
            sb_invp = []
            t0s = []
            t_acc = 0
            for s, ts in enumerate(SEGS):
                t0s.append(t_acc)
                tm = singles.tile([ts, P], F32, tag=f"invm{s}")
                nc.sync.dma_start(out=tm[:], in_=invc_m[t_acc:t_acc + ts, :])
                sb_invm.append(tm)
                tp_ = singles.tile([ts, P], F32, tag=f"invp{s}")
                nc.sync.dma_start(out=tp_[:], in_=invc_p[t_acc:t_acc + ts, :])
                sb_invp.append(tp_)
                t_acc += ts

            sb_eps = singles.tile([P, 1], F32)
            nc.vector.memset(sb_eps[:], EPS)
            carry = singles.tile([2, 1], F32)
            nc.vector.memset(carry[:], 0.0)

            bn = singles.tile([P, NT, 6], F32)   # per-row bn_stats
            inv_t = singles.tile([P, NT], F32)   # rstd, tile-major
            nmi_t = singles.tile([P, NT], F32)   # -mean*rstd, tile-major

            gamma_bc = sb_gamma[:].rearrange("p (o h) -> p o h", o=1).to_broadcast(
                (P, G, H)
            )
            if use_beta:
                beta_bc = sb_beta[:].rearrange("p (o h) -> p o h", o=1).to_broadcast(
                    (P, G, H)
                )

            xtiles = {}

            def load_seg(s):
                t0, ts = t0s[s], SEGS[s]
                for g0 in range(t0 // G, (t0 + ts) // G):
                    xt = xpool.tile([P, G, H], F32)
                    nc.sync.dma_start(out=xt[:], in_=xr[:, g0 * G:(g0 + 1) * G, :])
                    xtiles[g0] = xt
                    for j in range(G):
                        t = g0 * G + j
                        nc.vector.bn_stats(out=bn[:, t, :], in_=xt[:, j, :])

            def scan_seg(s):
                t0, ts = t0s[s], SEGS[s]
                t1 = t0 + ts
                me = bn[:, t0:t1, 1]
                mo = bn[:, t0:t1, 4]
                m2e = bn[:, t0:t1, 2]
                m2o = bn[:, t0:t1, 5]
                # 128*mean^2 planes (the only elementwise prep needed)
                pe = segp.tile([P, ts], F32, tag="pe")
                nc.vector.scalar_tensor_tensor(
                    out=pe[:], in0=me, scalar=128.0, in1=me,
                    op0=ALU.mult, op1=ALU.mult,
                )
                po = segp.tile([P, ts], F32, tag="po")
                nc.vector.scalar_tensor_tensor(
                    out=po[:], in0=mo, scalar=128.0, in1=mo,
                    op0=ALU.mult, op1=ALU.mult,
                )

                # merge + transpose fused on PE: chunk-major sums
                ps_s = psum.tile([ts, P], F32, tag="ps_s")
                nc.tensor.matmul(
                    ps_s[:], lhsT=me, rhs=sb_ident[:], start=True, stop=False
                )
                nc.tensor.matmul(
                    ps_s[:], lhsT=mo, rhs=sb_ident[:], start=False, stop=True
                )
                ps_p = psum.tile([ts, P], F32, tag="ps_p")
                nc.tensor.matmul(
                    ps_p[:], lhsT=pe[:], rhs=sb_ident[:], start=True, stop=False
                )
                nc.tensor.matmul(
                    ps_p[:], lhsT=po[:], rhs=sb_ident[:], start=False, stop=False
                )
                nc.tensor.matmul(
                    ps_p[:], lhsT=m2e, rhs=sb_ident[:], start=False, stop=False
                )
                nc.tensor.matmul(
                    ps_p[:], lhsT=m2o, rhs=sb_ident[:], start=False, stop=True
                )

                # within-chunk prefix scans (rowsum/128 and rowsumsq)
                scan_s = segp.tile([ts, P], F32, tag="scan_s")
                nc.vector.tensor_tensor_scan(
                    out=scan_s[:], data0=ps_s[:], data1=sb_invm[s][:],
                    initial=0.0, op0=ALU.add, op1=ALU.bypass,
                )
                scan_p = segp.tile([ts, P], F32, tag="scan_p")
                nc.vector.tensor_tensor_scan(
                    out=scan_p[:], data0=ps_p[:], data1=sb_invm[s][:],
                    initial=0.0, op0=ALU.add, op1=ALU.bypass,
                )

                # cross-chunk exclusive carry (seeded by inter-segment carry)
                tot = segp.tile([ts, 2], F32, tag="tot")
                nc.vector.tensor_copy(out=tot[:, 0:1], in_=scan_s[:, P - 1:P])
                nc.vector.tensor_copy(out=tot[:, 1:2], in_=scan_p[:, P - 1:P])
                pt = psum.tile([2, ts], F32, tag="pt")
                nc.tensor.transpose(pt[:], tot[:], sb_ident[0:ts, 0:ts])
                excl = segp.tile([2, ts], F32, tag="excl")
                nc.vector.tensor_copy(out=excl[:, 0:1], in_=carry[:])
                nc.vector.tensor_tensor_scan(
                    out=excl[:, 1:ts], data0=pt[:, 0:ts - 1],
                    data1=sb_invm[s][0:2, 0:ts - 1],
                    initial=carry[:], op0=ALU.add, op1=ALU.bypass,
                )
                # carry += segment total
                nc.vector.tensor_add(
                    out=carry[:], in0=excl[:, ts - 1:ts], in1=pt[:, ts - 1:ts],
                )
                ps_o = psum.tile([ts, 2], F32, tag="ps_o")
                nc.tensor.transpose(ps_o[:], excl[:], sb_ident[0:2, 0:2])

                # mean / msq / var / rstd / -mean*rstd  (chunk-major)
                mean_c = segp.tile([ts, P], F32, tag="mean_c")
                nc.vector.scalar_tensor_tensor(
                    out=mean_c[:], in0=scan_s[:], scalar=ps_o[:, 0:1],
                    in1=sb_invm[s][:], op0=ALU.add, op1=ALU.mult,
                )
                msq_c = segp.tile([ts, P], F32, tag="msq_c")
                nc.vector.scalar_tensor_tensor(
                    out=msq_c[:], in0=scan_p[:], scalar=ps_o[:, 1:2],
                    in1=sb_invp[s][:], op0=ALU.add, op1=ALU.mult,
                )
                var_c = segp.tile([ts, P], F32, tag="var_c")
                nc.vector.tensor_mul(out=var_c[:], in0=mean_c[:], in1=mean_c[:])
                nc.vector.tensor_sub(out=var_c[:], in0=msq_c[:], in1=var_c[:])
                sd_c = segp.tile([ts, P], F32, tag="sd_c")
                nc.scalar.activation(
                    out=sd_c[:], in_=var_c[:], func=ACTF.Sqrt,
                    bias=sb_eps[0:ts, :],
                )
                inv_c = segp.tile([ts, P], F32, tag="inv_c")
                nc.vector.reciprocal(out=inv_c[:], in_=sd_c[:])
                nmi_c = segp.tile([ts, P], F32, tag="nmi_c")
                nc.vector.scalar_tensor_tensor(
                    out=nmi_c[:], in0=mean_c[:], scalar=-1.0, in1=inv_c[:],
                    op0=ALU.mult, op1=ALU.mult,
                )

                # back to tile-major
                ps_inv = psum.tile([P, ts], F32, tag="ps_inv")
                nc.tensor.transpose(ps_inv[:], inv_c[:], sb_ident[0:ts, 0:ts])
                ps_nmi = psum.tile([P, ts], F32, tag="ps_nmi")
                nc.tensor.transpose(ps_nmi[:], nmi_c[:], sb_ident[0:ts, 0:ts])
                nc.scalar.copy(out=inv_t[:, t0:t1], in_=ps_inv[:])
                nc.scalar.copy(out=nmi_t[:, t0:t1], in_=ps_nmi[:])

            def out_seg(s):
                t0, ts = t0s[s], SEGS[s]
                for g0 in range(t0 // G, (t0 + ts) // G):
                    ob = opool.tile([P, G, H], F32)
                    xt = xtiles[g0]
                    use_gps_aff = g0 in AFFINE_GPS_GROUPS
                    for j in range(G):
                        t = g0 * G + j
                        if use_gps_aff:
                            nc.gpsimd.tensor_scalar(
                                out=ob[:, j, :], in0=xt[:, j, :],
                                scalar1=inv_t[:, t:t + 1],
                                scalar2=nmi_t[:, t:t + 1],
                                op0=ALU.mult, op1=ALU.add,
                            )
                        else:
                            nc.scalar.activation(
                                out=ob[:, j, :], in_=xt[:, j, :],
                                func=ACTF.Identity,
                                bias=nmi_t[:, t:t + 1], scale=inv_t[:, t:t + 1],
                            )
                    geng = nc.vector if g0 in GAMMA_DVE_GROUPS else nc.gpsimd
                    geng.tensor_mul(out=ob[:], in0=ob[:], in1=gamma_bc)
                    if use_beta:
                        geng.tensor_add(out=ob[:], in0=ob[:], in1=beta_bc)
                    # stores ride the ACT HWDGE queue so they never block loads
                    nc.scalar.dma_start(out=yr[:, g0 * G:(g0 + 1) * G, :], in_=ob[:])

            # software-pipelined emission: phase3 lags one segment
            nseg = len(SEGS)
            load_seg(0)
            scan_seg(0)
            for s in range(1, nseg):
                load_seg(s)
                out_seg(s - 1)
                scan_seg(s)
            out_seg(nseg - 1)

    nc.compile()
    return nc


_CACHE = {}


def _get(use_beta: bool):
    if use_beta not in _CACHE:
        _CACHE[use_beta] = _build(use_beta)
    return _CACHE[use_beta]


def _make_consts():
    ident = np.eye(P, dtype=np.float32)
    counts = np.arange(K, dtype=np.float64) + 1.0
    invc_m = (1.0 / (2.0 * counts)).reshape(NT, P).astype(np.float32)
    invc_p = (1.0 / (float(H) * counts)).reshape(NT, P).astype(np.float32)
    return ident, invc_m, invc_p


def _prepare(inputs, gamma, beta):
    inputs = np.ascontiguousarray(inputs, dtype=np.float32)
    gamma = np.asarray(gamma, dtype=np.float32).reshape(1, H)
    beta = np.asarray(beta, dtype=np.float32).reshape(1, H)
    use_beta = bool(np.any(beta))

    gamma_b = np.ascontiguousarray(np.broadcast_to(gamma, (P, H)))
    ident, invc_m, invc_p = _make_consts()

    in_maps = []
    for b in range(B):
        m = {
            "x": np.ascontiguousarray(inputs[b]),
            "gamma_b": gamma_b,
            "ident": ident,
            "invc_m": invc_m,
            "invc_p": invc_p,
        }
        if use_beta:
            m["beta_b"] = np.ascontiguousarray(np.broadcast_to(beta, (P, H)))
        in_maps.append(m)
    return use_beta, in_maps


def kernel(inputs: np.ndarray, gamma: np.ndarray, beta: np.ndarray) -> np.ndarray:
    use_beta, in_maps = _prepare(inputs, gamma, beta)
    nc = _get(use_beta)
    res = run_bass_kernel_spmd(nc, in_maps, list(range(B)))
    out = np.stack([res.results[b]["y"] for b in range(B)], axis=0)
    return out


# revision 11
# speedup vs baseline: 1.0760x; 1.0760x over previous
"""Causal (cumulative) LayerNorm Trainium2 Bass kernel.

Full-input contract: kernel(inputs, gamma, beta) takes the full
(B=8, K=16000, H=256) f32 tensor, shards batch across 8 NeuronCores
(one sample per core), and returns the full (8, 16000, 256) output.

Per-core algorithm (x is (K, H)):
  rowsum[k]   = sum_h x[k, h]
  rowsumsq[k] = sum_h x[k, h]^2
  csum = cumsum(rowsum); cpow = cumsum(rowsumsq)
  mean[k] = csum[k] / (H*(k+1));  msq[k] = cpow[k] / (H*(k+1))
  var[k] = msq[k] - mean[k]^2
  out[k, h] = gamma[h] * (x[k, h] - mean[k]) / sqrt(var[k] + EPS) + beta[h]

Layout: K = 16000 = 125 row-tiles x 128 rows, SBUF-resident tile-major
as (128 part, 125 tile, 256 h), row k = t*128 + p at [p, t, :].
Per-row sums come from one bn_stats per tile (even/odd mean/M2). The
even/odd merge and the tile-major -> chunk-major transpose are fused
into accumulated PE matmuls against an identity (PE fp32 is
full-precision; verified ~6e-8 rel): ps_s = me.T@I + mo.T@I,
ps_p = (128*me^2).T@I + (128*mo^2).T@I + m2e.T@I + m2o.T@I.

Tiles are processed in variable-size segments (small first segment so
the output pass starts early, small last segment for a short drain).
Per segment: chunk-major (ts, 128) scans along the free axis (fp32
vector scan), cross-chunk carry via a (2,)-wide transpose pair seeded
by a running inter-segment carry cell, per-row scale/bias (rstd,
-mean*rstd) transposed back tile-major. Output pass per group of 5
tiles: per-tile affine (scalar engine; a couple of groups on gpsimd),
batched gamma multiply (gpsimd mostly, some vector), store.

Loads ride the SP HWDGE queue and stores the Activation HWDGE queue so
a store waiting on compute never blocks later loads. DMA is the
roofline: ~32.8 MB @ ~358 GB/s/core.
"""

import numpy as np

import concourse.bass as bass
import concourse.bacc as bacc
import concourse.tile as tile
from concourse import mybir
from concourse.bass_utils import run_bass_kernel_spmd

EPS = 1e-8
B, K, H = 8, 16000, 256
P = 128                 # SBUF partitions
NT = K // P             # 125 row-tiles per sample
G = 5                   # tiles per DMA group / output group
NG = NT // G            # 25 groups
SEGS = [10, 25, 30, 30, 20, 10]      # tiles per scan segment (sum = NT)
assert sum(SEGS) == NT and all(ts % G == 0 for ts in SEGS)
F32 = mybir.dt.float32
ALU = mybir.AluOpType
ACTF = mybir.ActivationFunctionType

GAMMA_DVE_GROUPS = {0, 5, 10, 15, 20}   # gamma-mult on vector, rest gpsimd
AFFINE_GPS_GROUPS = {7, 15}             # affine on gpsimd, rest scalar


def _build(use_beta: bool):
    nc = bacc.Bacc("TRN2", target_bir_lowering=False, debug=False)

    x = nc.declare_dram_parameter("x", [K, H], F32, isOutput=False)
    gamma_b = nc.declare_dram_parameter("gamma_b", [P, H], F32, isOutput=False)
    beta_b = (
        nc.declare_dram_parameter("beta_b", [P, H], F32, isOutput=False)
        if use_beta
        else None
    )
    ident = nc.declare_dram_parameter("ident", [P, P], F32, isOutput=False)
    invc_m = nc.declare_dram_parameter("invc_m", [NT, P], F32, isOutput=False)
    invc_p = nc.declare_dram_parameter("invc_p", [NT, P], F32, isOutput=False)
    y = nc.declare_dram_parameter("y", [K, H], F32, isOutput=True)

    xr = x.rearrange("(t p) h -> p t h", p=P)   # [128, 125, 256]
    yr = y.rearrange("(t p) h -> p t h", p=P)

    with tile.TileContext(nc) as tc:
        with (
            tc.tile_pool(name="singles", bufs=1) as singles,
            tc.tile_pool(name="xpool", bufs=NG) as xpool,
            tc.tile_pool(name="opool", bufs=8) as opool,
            tc.tile_pool(name="segp", bufs=2) as segp,
            tc.tile_pool(name="psum", bufs=1, space="PSUM") as psum,
        ):
            sb_gamma = singles.tile([P, H], F32)
            nc.sync.dma_start(out=sb_gamma[:], in_=gamma_b[:])
            if use_beta:
                sb_beta = singles.tile([P, H], F32)
                nc.sync.dma_start(out=sb_beta[:], in_=beta_b[:])
            sb_ident = singles.tile([P, P], F32)
            nc.sync.dma_start(out=sb_ident[:], in_=ident[:])

            sb_invm = []
            sb_invp = []
            t0s = []
            t_acc = 0
            for s, ts in enumerate(SEGS):
                t0s.append(t_acc)
                tm = singles.tile([ts, P], F32, tag=f"invm{s}")
                nc.sync.dma_start(out=tm[:], in_=invc_m[t_acc:t_acc + ts, :])
                sb_invm.append(tm)
                tp_ = singles.tile([ts, P], F32, tag=f"invp{s}")
                nc.sync.dma_start(out=tp_[:], in_=invc_p[t_acc:t_acc + ts, :])
                sb_invp.append(tp_)
                t_acc += ts

            sb_eps = singles.tile([P, 1], F32)
            nc.vector.memset(sb_eps[:], EPS)
            carry = singles.tile([2, 1], F32)
            nc.vector.memset(carry[:], 0.0)

            bn = singles.tile([P, NT, 6], F32)   # per-row bn_stats
            inv_t = singles.tile([P, NT], F32)   # rstd, tile-major
            nmi_t = singles.tile([P, NT], F32)   # -mean*rstd, tile-major

            gamma_bc = sb_gamma[:].rearrange("p (o h) -> p o h", o=1).to_broadcast(
                (P, G, H)
            )
            if use_beta:
                beta_bc = sb_beta[:].rearrange("p (o h) -> p o h", o=1).to_broadcast(
                    (P, G, H)
                )

            xtiles = {}

            def load_seg(s):
                t0, ts = t0s[s], SEGS[s]
                for g0 in range(t0 // G, (t0 + ts) // G):
                    xt = xpool.tile([P, G, H], F32)
                    nc.sync.dma_start(out=xt[:], in_=xr[:, g0 * G:(g0 + 1) * G, :])
                    xtiles[g0] = xt
                    for j in range(G):
                        t = g0 * G + j
                        nc.vector.bn_stats(out=bn[:, t, :], in_=xt[:, j, :])

            def scan_seg(s):
                t0, ts = t0s[s], SEGS[s]
                t1 = t0 + ts
                me = bn[:, t0:t1, 1]
                mo = bn[:, t0:t1, 4]
                m2e = bn[:, t0:t1, 2]
                m2o = bn[:, t0:t1, 5]
                # 128*mean^2 planes (the only elementwise prep needed)
                pe = segp.tile([P, ts], F32, tag="pe")
                nc.vector.scalar_tensor_tensor(
                    out=pe[:], in0=me, scalar=128.0, in1=me,
                    op0=ALU.mult, op1=ALU.mult,
                )
                po = segp.tile([P, ts], F32, tag="po")
                nc.vector.scalar_tensor_tensor(
                    out=po[:], in0=mo, scalar=128.0, in1=mo,
                    op0=ALU.mult, op1=ALU.mult,
                )

                # merge + transpose fused on PE: chunk-major sums
                ps_s = psum.tile([ts, P], F32, tag="ps_s")
                nc.tensor.matmul(
                    ps_s[:], lhsT=me, rhs=sb_ident[:], start=True, stop=False
                )
                nc.tensor.matmul(
                    ps_s[:], lhsT=mo, rhs=sb_ident[:], start=False, stop=True
                )
                ps_p = psum.tile([ts, P], F32, tag="ps_p")
                nc.tensor.matmul(
                    ps_p[:], lhsT=pe[:], rhs=sb_ident[:], start=True, stop=False
                )
                nc.tensor.matmul(
                    ps_p[:], lhsT=po[:], rhs=sb_ident[:], start=False, stop=False
                )
                nc.tensor.matmul(
                    ps_p[:], lhsT=m2e, rhs=sb_ident[:], start=False, stop=False
                )
                nc.tensor.matmul(
                    ps_p[:], lhsT=m2o, rhs=sb_ident[:], start=False, stop=True
                )

                # within-chunk prefix scans (rowsum/128 and rowsumsq)
                scan_s = segp.tile([ts, P], F32, tag="scan_s")
                nc.vector.tensor_tensor_scan(
                    out=scan_s[:], data0=ps_s[:], data1=sb_invm[s][:],
                    initial=0.0, op0=ALU.add, op1=ALU.bypass,
                )
                scan_p = segp.tile([ts, P], F32, tag="scan_p")
                nc.vector.tensor_tensor_scan(
                    out=scan_p[:], data0=ps_p[:], data1=sb_invm[s][:],
                    initial=0.0, op0=ALU.add, op1=ALU.bypass,
                )

                # cross-chunk exclusive carry (seeded by inter-segment carry)
                tot = segp.tile([ts, 2], F32, tag="tot")
                nc.vector.tensor_copy(out=tot[:, 0:1], in_=scan_s[:, P - 1:P])
                nc.vector.tensor_copy(out=tot[:, 1:2], in_=scan_p[:, P - 1:P])
                pt = psum.tile([2, ts], F32, tag="pt")
                nc.tensor.transpose(pt[:], tot[:], sb_ident[0:ts, 0:ts])
                excl = segp.tile([2, ts], F32, tag="excl")
                nc.vector.tensor_copy(out=excl[:, 0:1], in_=carry[:])
                nc.vector.tensor_tensor_scan(
                    out=excl[:, 1:ts], data0=pt[:, 0:ts - 1],
                    data1=sb_invm[s][0:2, 0:ts - 1],
                    initial=carry[:], op0=ALU.add, op1=ALU.bypass,
                )
                # carry += segment total
                nc.vector.tensor_add(
                    out=carry[:], in0=excl[:, ts - 1:ts], in1=pt[:, ts - 1:ts],
                )
                ps_o = psum.tile([ts, 2], F32, tag="ps_o")
                nc.tensor.transpose(ps_o[:], excl[:], sb_ident[0:2, 0:2])

                # mean / msq / var / rstd / -mean*rstd  (chunk-major)
                mean_c = segp.tile([ts, P], F32, tag="mean_c")
                nc.vector.scalar_tensor_tensor(
                    out=mean_c[:], in0=scan_s[:], scalar=ps_o[:, 0:1],
                    in1=sb_invm[s][:], op0=ALU.add, op1=ALU.mult,
                )
                msq_c = segp.tile([ts, P], F32, tag="msq_c")
                nc.vector.scalar_tensor_tensor(
                    out=msq_c[:], in0=scan_p[:], scalar=ps_o[:, 1:2],
                    in1=sb_invp[s][:], op0=ALU.add, op1=ALU.mult,
                )
                var_c = segp.tile([ts, P], F32, tag="var_c")
                nc.vector.tensor_mul(out=var_c[:], in0=mean_c[:], in1=mean_c[:])
                nc.vector.tensor_sub(out=var_c[:], in0=msq_c[:], in1=var_c[:])
                sd_c = segp.tile([ts, P], F32, tag="sd_c")
                nc.scalar.activation(
                    out=sd_c[:], in_=var_c[:], func=ACTF.Sqrt,
                    bias=sb_eps[0:ts, :],
                )
                inv_c = segp.tile([ts, P], F32, tag="inv_c")
                nc.vector.reciprocal(out=inv_c[:], in_=sd_c[:])
                nmi_c = segp.tile([ts, P], F32, tag="nmi_c")
                nc.vector.scalar_tensor_tensor(
                    out=nmi_c[:], in0=mean_c[:], scalar=-1.0, in1=inv_c[:],
                    op0=ALU.mult, op1=ALU.mult,
                )

                # back to tile-major
                ps_inv = psum.tile([P, ts], F32, tag="ps_inv")
                nc.tensor.transpose(ps_inv[:], inv_c[:], sb_ident[0:ts, 0:ts])
                ps_nmi = psum.tile([P, ts], F32, tag="ps_nmi")
                nc.tensor.transpose(ps_nmi[:], nmi_c[:], sb_ident[0:ts, 0:ts])
                nc.scalar.copy(out=inv_t[:, t0:t1], in_=ps_inv[:])
                nc.scalar.copy(out=nmi_t[:, t0:t1], in_=ps_nmi[:])

            def out_seg(s):
                t0, ts = t0s[s], SEGS[s]
                obs = []
                for g0 in range(t0 // G, (t0 + ts) // G):
                    ob = opool.tile([P, G, H], F32)
                    xt = xtiles[g0]
                    use_gps_aff = g0 in AFFINE_GPS_GROUPS
                    for j in range(G):
                        t = g0 * G + j
                        if use_gps_aff:
                            nc.gpsimd.tensor_scalar(
                                out=ob[:, j, :], in0=xt[:, j, :],
                                scalar1=inv_t[:, t:t + 1],
                                scalar2=nmi_t[:, t:t + 1],
                                op0=ALU.mult, op1=ALU.add,
                            )
                        else:
                            nc.scalar.activation(
                                out=ob[:, j, :], in_=xt[:, j, :],
                                func=ACTF.Identity,
                                bias=nmi_t[:, t:t + 1], scale=inv_t[:, t:t + 1],
                            )
                    geng = nc.vector if g0 in GAMMA_DVE_GROUPS else nc.gpsimd
                    geng.tensor_mul(out=ob[:], in0=ob[:], in1=gamma_bc)
                    if use_beta:
                        geng.tensor_add(out=ob[:], in0=ob[:], in1=beta_bc)
                    obs.append((g0, ob))
                # stores ride the ACT HWDGE queue, emitted after ALL the
                # segment's affines so a trigger waiting on a gamma-mult
                # never stalls later affine work in the ACT stream
                for g0, ob in obs:
                    nc.scalar.dma_start(out=yr[:, g0 * G:(g0 + 1) * G, :], in_=ob[:])

            # software-pipelined emission: phase3 lags one segment
            nseg = len(SEGS)
            load_seg(0)
            scan_seg(0)
            for s in range(1, nseg):
                load_seg(s)
                out_seg(s - 1)
                scan_seg(s)
            out_seg(nseg - 1)

    nc.compile()
    return nc


_CACHE = {}


def _get(use_beta: bool):
    if use_beta not in _CACHE:
        _CACHE[use_beta] = _build(use_beta)
    return _CACHE[use_beta]


def _make_consts():
    ident = np.eye(P, dtype=np.float32)
    counts = np.arange(K, dtype=np.float64) + 1.0
    invc_m = (1.0 / (2.0 * counts)).reshape(NT, P).astype(np.float32)
    invc_p = (1.0 / (float(H) * counts)).reshape(NT, P).astype(np.float32)
    return ident, invc_m, invc_p


def _prepare(inputs, gamma, beta):
    inputs = np.ascontiguousarray(inputs, dtype=np.float32)
    gamma = np.asarray(gamma, dtype=np.float32).reshape(1, H)
    beta = np.asarray(beta, dtype=np.float32).reshape(1, H)
    use_beta = bool(np.any(beta))

    gamma_b = np.ascontiguousarray(np.broadcast_to(gamma, (P, H)))
    ident, invc_m, invc_p = _make_consts()

    in_maps = []
    for b in range(B):
        m = {
            "x": np.ascontiguousarray(inputs[b]),
            "gamma_b": gamma_b,
            "ident": ident,
            "invc_m": invc_m,
            "invc_p": invc_p,
        }
        if use_beta:
            m["beta_b"] = np.ascontiguousarray(np.broadcast_to(beta, (P, H)))
        in_maps.append(m)
    return use_beta, in_maps


def kernel(inputs: np.ndarray, gamma: np.ndarray, beta: np.ndarray) -> np.ndarray:
    use_beta, in_maps = _prepare(inputs, gamma, beta)
    nc = _get(use_beta)
    res = run_bass_kernel_spmd(nc, in_maps, list(range(B)))
    out = np.stack([res.results[b]["y"] for b in range(B)], axis=0)
    return out


# revision 12
# speedup vs baseline: 1.2053x; 1.1201x over previous
"""Causal (cumulative) LayerNorm Trainium2 Bass kernel.

Full-input contract: kernel(inputs, gamma, beta) takes the full
(B=8, K=16000, H=256) f32 tensor, shards batch across 8 NeuronCores
(one sample per core), and returns the full (8, 16000, 256) output.

Per-core algorithm (x is (K, H)):
  rowsum[k]   = sum_h x[k, h]
  rowsumsq[k] = sum_h x[k, h]^2
  csum = cumsum(rowsum); cpow = cumsum(rowsumsq)
  mean[k] = csum[k] / (H*(k+1));  msq[k] = cpow[k] / (H*(k+1))
  var[k] = msq[k] - mean[k]^2
  out[k, h] = gamma[h] * (x[k, h] - mean[k]) / sqrt(var[k] + EPS) + beta[h]

Layout: K = 16000 = 125 row-tiles x 128 rows, SBUF-resident tile-major
as (128 part, 125 tile, 256 h), row k = t*128 + p at [p, t, :].
Per-row sums come from one bn_stats per tile (even/odd mean/M2). The
even/odd merge and the tile-major -> chunk-major transpose are fused
into accumulated PE matmuls against an identity (PE fp32 is
full-precision; verified ~6e-8 rel): ps_s = me.T@I + mo.T@I,
ps_p = (128*me^2).T@I + (128*mo^2).T@I + m2e.T@I + m2o.T@I.

Tiles are processed in variable-size segments (small first segment so
the output pass starts early, small last segment for a short drain).
Per segment: chunk-major (ts, 128) scans along the free axis (fp32
vector scan), cross-chunk carry via a (2,)-wide transpose pair seeded
by a running inter-segment carry cell, per-row scale/bias (rstd,
-mean*rstd) transposed back tile-major. Output pass per group of 5
tiles: per-tile affine (scalar engine; a couple of groups on gpsimd),
batched gamma multiply (gpsimd mostly, some vector), store.

Loads ride the SP HWDGE queue and stores the Activation HWDGE queue so
a store waiting on compute never blocks later loads. DMA is the
roofline: ~32.8 MB @ ~358 GB/s/core.
"""

import numpy as np

import concourse.bass as bass
import concourse.bacc as bacc
import concourse.tile as tile
from concourse import mybir
from concourse.bass_utils import run_bass_kernel_spmd

EPS = 1e-8
B, K, H = 8, 16000, 256
P = 128                 # SBUF partitions
NT = K // P             # 125 row-tiles per sample
G = 5                   # tiles per DMA group / output group
NG = NT // G            # 25 groups
SEGS = [10, 25, 30, 30, 20, 10]      # tiles per scan segment (sum = NT)
assert sum(SEGS) == NT and all(ts % G == 0 for ts in SEGS)
F32 = mybir.dt.float32
ALU = mybir.AluOpType
ACTF = mybir.ActivationFunctionType

GAMMA_DVE_GROUPS = {0, 5, 10, 15, 20}   # gamma-mult on vector, rest gpsimd
AFFINE_GPS_GROUPS = {7, 15}             # affine on gpsimd, rest scalar


def _build(use_beta: bool):
    nc = bacc.Bacc("TRN2", target_bir_lowering=False, debug=False)

    x = nc.declare_dram_parameter("x", [K, H], F32, isOutput=False)
    gamma_b = nc.declare_dram_parameter("gamma_b", [P, H], F32, isOutput=False)
    beta_b = (
        nc.declare_dram_parameter("beta_b", [P, H], F32, isOutput=False)
        if use_beta
        else None
    )
    ident = nc.declare_dram_parameter("ident", [P, P], F32, isOutput=False)
    invc_m = nc.declare_dram_parameter("invc_m", [NT, P], F32, isOutput=False)
    invc_p = nc.declare_dram_parameter("invc_p", [NT, P], F32, isOutput=False)
    y = nc.declare_dram_parameter("y", [K, H], F32, isOutput=True)

    xr = x.rearrange("(t p) h -> p t h", p=P)   # [128, 125, 256]
    yr = y.rearrange("(t p) h -> p t h", p=P)

    with tile.TileContext(nc) as tc:
        with (
            tc.tile_pool(name="singles", bufs=1) as singles,
            tc.tile_pool(name="xpool", bufs=NG) as xpool,
            tc.tile_pool(name="opool", bufs=8) as opool,
            tc.tile_pool(name="segp", bufs=2) as segp,
            tc.tile_pool(name="psum", bufs=1, space="PSUM") as psum,
        ):
            sb_gamma = singles.tile([P, H], F32)
            nc.sync.dma_start(out=sb_gamma[:], in_=gamma_b[:])
            if use_beta:
                sb_beta = singles.tile([P, H], F32)
                nc.sync.dma_start(out=sb_beta[:], in_=beta_b[:])
            sb_ident = singles.tile([P, P], F32)
            nc.sync.dma_start(out=sb_ident[:], in_=ident[:])

            sb_invm = []
            sb_invp = []
            t0s = []
            t_acc = 0
            for s, ts in enumerate(SEGS):
                t0s.append(t_acc)
                tm = singles.tile([ts, P], F32, tag=f"invm{s}")
                nc.sync.dma_start(out=tm[:], in_=invc_m[t_acc:t_acc + ts, :])
                sb_invm.append(tm)
                tp_ = singles.tile([ts, P], F32, tag=f"invp{s}")
                nc.sync.dma_start(out=tp_[:], in_=invc_p[t_acc:t_acc + ts, :])
                sb_invp.append(tp_)
                t_acc += ts

            sb_eps = singles.tile([P, 1], F32)
            nc.vector.memset(sb_eps[:], EPS)
            carry = singles.tile([2, 1], F32)
            nc.vector.memset(carry[:], 0.0)

            bn = singles.tile([P, NT, 6], F32)   # per-row bn_stats
            inv_t = singles.tile([P, NT], F32)   # rstd, tile-major
            nmi_t = singles.tile([P, NT], F32)   # -mean*rstd, tile-major

            gamma_bc = sb_gamma[:].rearrange("p (o h) -> p o h", o=1).to_broadcast(
                (P, G, H)
            )
            if use_beta:
                beta_bc = sb_beta[:].rearrange("p (o h) -> p o h", o=1).to_broadcast(
                    (P, G, H)
                )

            xtiles = {}

            def load_seg(s):
                t0, ts = t0s[s], SEGS[s]
                for g0 in range(t0 // G, (t0 + ts) // G):
                    xt = xpool.tile([P, G, H], F32)
                    nc.sync.dma_start(out=xt[:], in_=xr[:, g0 * G:(g0 + 1) * G, :])
                    xtiles[g0] = xt
                    for j in range(G):
                        t = g0 * G + j
                        nc.vector.bn_stats(out=bn[:, t, :], in_=xt[:, j, :])

            def scan_seg(s):
                t0, ts = t0s[s], SEGS[s]
                t1 = t0 + ts
                me = bn[:, t0:t1, 1]
                mo = bn[:, t0:t1, 4]
                m2e = bn[:, t0:t1, 2]
                m2o = bn[:, t0:t1, 5]
                # 128*mean^2 planes (the only elementwise prep needed)
                pe = segp.tile([P, ts], F32, tag="pe")
                nc.vector.scalar_tensor_tensor(
                    out=pe[:], in0=me, scalar=128.0, in1=me,
                    op0=ALU.mult, op1=ALU.mult,
                )
                po = segp.tile([P, ts], F32, tag="po")
                nc.vector.scalar_tensor_tensor(
                    out=po[:], in0=mo, scalar=128.0, in1=mo,
                    op0=ALU.mult, op1=ALU.mult,
                )

                # merge + transpose fused on PE: chunk-major sums
                ps_s = psum.tile([ts, P], F32, tag="ps_s")
                nc.tensor.matmul(
                    ps_s[:], lhsT=me, rhs=sb_ident[:], start=True, stop=False
                )
                nc.tensor.matmul(
                    ps_s[:], lhsT=mo, rhs=sb_ident[:], start=False, stop=True
                )
                ps_p = psum.tile([ts, P], F32, tag="ps_p")
                nc.tensor.matmul(
                    ps_p[:], lhsT=pe[:], rhs=sb_ident[:], start=True, stop=False
                )
                nc.tensor.matmul(
                    ps_p[:], lhsT=po[:], rhs=sb_ident[:], start=False, stop=False
                )
                nc.tensor.matmul(
                    ps_p[:], lhsT=m2e, rhs=sb_ident[:], start=False, stop=False
                )
                nc.tensor.matmul(
                    ps_p[:], lhsT=m2o, rhs=sb_ident[:], start=False, stop=True
                )

                # within-chunk prefix scans (rowsum/128 and rowsumsq)
                scan_s = segp.tile([ts, P], F32, tag="scan_s")
                nc.vector.tensor_tensor_scan(
                    out=scan_s[:], data0=ps_s[:], data1=sb_invm[s][:],
                    initial=0.0, op0=ALU.add, op1=ALU.bypass,
                )
                scan_p = segp.tile([ts, P], F32, tag="scan_p")
                nc.vector.tensor_tensor_scan(
                    out=scan_p[:], data0=ps_p[:], data1=sb_invm[s][:],
                    initial=0.0, op0=ALU.add, op1=ALU.bypass,
                )

                # cross-chunk exclusive carry (seeded by inter-segment carry)
                tot = segp.tile([ts, 2], F32, tag="tot")
                nc.vector.tensor_copy(out=tot[:, 0:1], in_=scan_s[:, P - 1:P])
                nc.vector.tensor_copy(out=tot[:, 1:2], in_=scan_p[:, P - 1:P])
                pt = psum.tile([2, ts], F32, tag="pt")
                nc.tensor.transpose(pt[:], tot[:], sb_ident[0:ts, 0:ts])
                excl = segp.tile([2, ts], F32, tag="excl")
                nc.vector.tensor_copy(out=excl[:, 0:1], in_=carry[:])
                nc.vector.tensor_tensor_scan(
                    out=excl[:, 1:ts], data0=pt[:, 0:ts - 1],
                    data1=sb_invm[s][0:2, 0:ts - 1],
                    initial=carry[:], op0=ALU.add, op1=ALU.bypass,
                )
                # carry += segment total
                nc.vector.tensor_add(
                    out=carry[:], in0=excl[:, ts - 1:ts], in1=pt[:, ts - 1:ts],
                )
                ps_o = psum.tile([ts, 2], F32, tag="ps_o")
                nc.tensor.transpose(ps_o[:], excl[:], sb_ident[0:2, 0:2])

                # mean / msq / var / rstd / -mean*rstd  (chunk-major)
                mean_c = segp.tile([ts, P], F32, tag="mean_c")
                nc.vector.scalar_tensor_tensor(
                    out=mean_c[:], in0=scan_s[:], scalar=ps_o[:, 0:1],
                    in1=sb_invm[s][:], op0=ALU.add, op1=ALU.mult,
                )
                msq_c = segp.tile([ts, P], F32, tag="msq_c")
                nc.vector.scalar_tensor_tensor(
                    out=msq_c[:], in0=scan_p[:], scalar=ps_o[:, 1:2],
                    in1=sb_invp[s][:], op0=ALU.add, op1=ALU.mult,
                )
                var_c = segp.tile([ts, P], F32, tag="var_c")
                nc.vector.tensor_mul(out=var_c[:], in0=mean_c[:], in1=mean_c[:])
                nc.vector.tensor_sub(out=var_c[:], in0=msq_c[:], in1=var_c[:])
                sd_c = segp.tile([ts, P], F32, tag="sd_c")
                nc.scalar.activation(
                    out=sd_c[:], in_=var_c[:], func=ACTF.Sqrt,
                    bias=sb_eps[0:ts, :],
                )
                inv_c = segp.tile([ts, P], F32, tag="inv_c")
                nc.vector.reciprocal(out=inv_c[:], in_=sd_c[:])
                nmi_c = segp.tile([ts, P], F32, tag="nmi_c")
                nc.vector.scalar_tensor_tensor(
                    out=nmi_c[:], in0=mean_c[:], scalar=-1.0, in1=inv_c[:],
                    op0=ALU.mult, op1=ALU.mult,
                )

                # back to tile-major
                ps_inv = psum.tile([P, ts], F32, tag="ps_inv")
                nc.tensor.transpose(ps_inv[:], inv_c[:], sb_ident[0:ts, 0:ts])
                ps_nmi = psum.tile([P, ts], F32, tag="ps_nmi")
                nc.tensor.transpose(ps_nmi[:], nmi_c[:], sb_ident[0:ts, 0:ts])
                nc.scalar.copy(out=inv_t[:, t0:t1], in_=ps_inv[:])
                nc.scalar.copy(out=nmi_t[:, t0:t1], in_=ps_nmi[:])

            def out_seg(s):
                t0, ts = t0s[s], SEGS[s]
                obs = []
                for g0 in range(t0 // G, (t0 + ts) // G):
                    ob = opool.tile([P, G, H], F32)
                    xt = xtiles[g0]
                    use_gps_aff = g0 in AFFINE_GPS_GROUPS
                    for j in range(G):
                        t = g0 * G + j
                        if use_gps_aff:
                            nc.gpsimd.tensor_scalar(
                                out=ob[:, j, :], in0=xt[:, j, :],
                                scalar1=inv_t[:, t:t + 1],
                                scalar2=nmi_t[:, t:t + 1],
                                op0=ALU.mult, op1=ALU.add,
                            )
                        else:
                            nc.scalar.activation(
                                out=ob[:, j, :], in_=xt[:, j, :],
                                func=ACTF.Identity,
                                bias=nmi_t[:, t:t + 1], scale=inv_t[:, t:t + 1],
                            )
                    geng = nc.vector if g0 in GAMMA_DVE_GROUPS else nc.gpsimd
                    geng.tensor_mul(out=ob[:], in0=ob[:], in1=gamma_bc)
                    if use_beta:
                        geng.tensor_add(out=ob[:], in0=ob[:], in1=beta_bc)
                    obs.append((g0, ob))
                return obs

            def store_seg(obs):
                # stores ride the ACT HWDGE queue, emitted after the NEXT
                # segment's chain ops so a trigger waiting on a gamma-mult
                # never stalls the chain or later affines in the ACT stream
                for g0, ob in obs:
                    nc.scalar.dma_start(out=yr[:, g0 * G:(g0 + 1) * G, :], in_=ob[:])

            # software-pipelined emission: phase3 lags one segment, and
            # stores lag until after the next segment's chain emission
            nseg = len(SEGS)
            load_seg(0)
            scan_seg(0)
            pending = None
            for s in range(1, nseg):
                load_seg(s)
                obs = out_seg(s - 1)
                scan_seg(s)
                if pending:
                    store_seg(pending)
                pending = obs
            obs = out_seg(nseg - 1)
            if pending:
                store_seg(pending)
            store_seg(obs)

    nc.compile()
    return nc


_CACHE = {}


def _get(use_beta: bool):
    if use_beta not in _CACHE:
        _CACHE[use_beta] = _build(use_beta)
    return _CACHE[use_beta]


def _make_consts():
    ident = np.eye(P, dtype=np.float32)
    counts = np.arange(K, dtype=np.float64) + 1.0
    invc_m = (1.0 / (2.0 * counts)).reshape(NT, P).astype(np.float32)
    invc_p = (1.0 / (float(H) * counts)).reshape(NT, P).astype(np.float32)
    return ident, invc_m, invc_p


def _prepare(inputs, gamma, beta):
    inputs = np.ascontiguousarray(inputs, dtype=np.float32)
    gamma = np.asarray(gamma, dtype=np.float32).reshape(1, H)
    beta = np.asarray(beta, dtype=np.float32).reshape(1, H)
    use_beta = bool(np.any(beta))

    gamma_b = np.ascontiguousarray(np.broadcast_to(gamma, (P, H)))
    ident, invc_m, invc_p = _make_consts()

    in_maps = []
    for b in range(B):
        m = {
            "x": np.ascontiguousarray(inputs[b]),
            "gamma_b": gamma_b,
            "ident": ident,
            "invc_m": invc_m,
            "invc_p": invc_p,
        }
        if use_beta:
            m["beta_b"] = np.ascontiguousarray(np.broadcast_to(beta, (P, H)))
        in_maps.append(m)
    return use_beta, in_maps


def kernel(inputs: np.ndarray, gamma: np.ndarray, beta: np.ndarray) -> np.ndarray:
    use_beta, in_maps = _prepare(inputs, gamma, beta)
    nc = _get(use_beta)
    res = run_bass_kernel_spmd(nc, in_maps, list(range(B)))
    out = np.stack([res.results[b]["y"] for b in range(B)], axis=0)
    return out


# revision 13
# speedup vs baseline: 1.2435x; 1.0317x over previous
"""Causal (cumulative) LayerNorm Trainium2 Bass kernel.

Full-input contract: kernel(inputs, gamma, beta) takes the full
(B=8, K=16000, H=256) f32 tensor, shards batch across 8 NeuronCores
(one sample per core), and returns the full (8, 16000, 256) output.

Per-core algorithm (x is (K, H)):
  rowsum[k]   = sum_h x[k, h]
  rowsumsq[k] = sum_h x[k, h]^2
  csum = cumsum(rowsum); cpow = cumsum(rowsumsq)
  mean[k] = csum[k] / (H*(k+1));  msq[k] = cpow[k] / (H*(k+1))
  var[k] = msq[k] - mean[k]^2
  out[k, h] = gamma[h] * (x[k, h] - mean[k]) / sqrt(var[k] + EPS) + beta[h]

Layout: row k = b*3200 + p*25 + r for band b in 0..4, partition p in
0..127, r in 0..24. Each band is one (128, 25, 256) SBUF tile whose
per-partition 25 rows are CONTIGUOUS in HBM, so band loads/stores are
single DMA triggers with 25 KB contiguous runs per partition (max DMA
efficiency, negligible descriptor-gen on the sequencers).

Per band: one bn_stats per r (even/odd mean/M2 per row), six cheap
full-partition merges to rowsum/128 and rowsumsq, a fp32 vector scan
along r (prefix within each 25-row chunk), chunk totals prefix-summed
ACROSS partitions by one strictly-triangular PE matmul (PE fp32 is
full-precision, ~6e-8 verified) plus a broadcast matmul seeding the
running inter-band carry, then per-row rstd / -mean*rstd computed
directly in affine-ready layout (no transposes anywhere). Output pass
runs IN PLACE over the band tile: per-r affine (scalar engine mostly),
batched gamma multiply (gpsimd/vector), one store trigger per r-group.
Bands pipeline: band b's output overlaps band b+1's load. DMA is the
roofline: ~32.8 MB @ ~358 GB/s/core.
"""

import numpy as np

import concourse.bass as bass
import concourse.bacc as bacc
import concourse.tile as tile
from concourse import mybir
from concourse.bass_utils import run_bass_kernel_spmd

EPS = 1e-8
B, K, H = 8, 16000, 256
P = 128                  # SBUF partitions = chunks per band
CL = 25                  # rows per chunk (per partition per band)
BANDS = K // (P * CL)    # 5
G = 5                    # rows per gamma/store group
NGB = CL // G            # 5 groups per band
F32 = mybir.dt.float32
ALU = mybir.AluOpType
ACTF = mybir.ActivationFunctionType

GAMMA_DVE_J = {2}        # per-band group index -> gamma on vector, rest gpsimd
AFFINE_GPS_J = {1, 3}    # per-band group index -> affine on gpsimd, rest scalar


def _build(use_beta: bool):
    nc = bacc.Bacc("TRN2", target_bir_lowering=False, debug=False)

    x = nc.declare_dram_parameter("x", [K, H], F32, isOutput=False)
    gamma_b = nc.declare_dram_parameter("gamma_b", [P, H], F32, isOutput=False)
    beta_b = (
        nc.declare_dram_parameter("beta_b", [P, H], F32, isOutput=False)
        if use_beta
        else None
    )
    utri = nc.declare_dram_parameter("utri", [P, P], F32, isOutput=False)
    ident = nc.declare_dram_parameter("ident", [P, P], F32, isOutput=False)
    ones_col = nc.declare_dram_parameter("ones_col", [P, 1], F32, isOutput=False)
    ones_row = nc.declare_dram_parameter("ones_row", [1, P], F32, isOutput=False)
    invc_m = nc.declare_dram_parameter("invc_m", [P, BANDS, CL], F32, isOutput=False)
    invc_p = nc.declare_dram_parameter("invc_p", [P, BANDS, CL], F32, isOutput=False)
    y = nc.declare_dram_parameter("y", [K, H], F32, isOutput=True)

    xr = x.rearrange("(b p r) h -> b p r h", p=P, r=CL)   # [5, 128, 25, 256]
    yr = y.rearrange("(b p r) h -> b p r h", p=P, r=CL)

    with tile.TileContext(nc) as tc:
        with (
            tc.tile_pool(name="singles", bufs=1) as singles,
            tc.tile_pool(name="xband", bufs=BANDS) as xband,
            tc.tile_pool(name="opool", bufs=8) as opool,
            tc.tile_pool(name="segp", bufs=2) as segp,
            tc.tile_pool(name="psum", bufs=2, space="PSUM") as psum,
        ):
            sb_gamma = singles.tile([P, H], F32)
            nc.sync.dma_start(out=sb_gamma[:], in_=gamma_b[:])
            if use_beta:
                sb_beta = singles.tile([P, H], F32)
                nc.sync.dma_start(out=sb_beta[:], in_=beta_b[:])
            sb_utri = singles.tile([P, P], F32)
            nc.sync.dma_start(out=sb_utri[:], in_=utri[:])
            sb_ident = singles.tile([P, P], F32)
            nc.sync.dma_start(out=sb_ident[:], in_=ident[:])
            sb_onec = singles.tile([P, 1], F32)
            nc.sync.dma_start(out=sb_onec[:], in_=ones_col[:])
            sb_oner = singles.tile([1, P], F32)
            nc.sync.dma_start(out=sb_oner[:], in_=ones_row[:])
            sb_invm = singles.tile([P, BANDS, CL], F32)
            nc.sync.dma_start(out=sb_invm[:], in_=invc_m[:])
            sb_invp = singles.tile([P, BANDS, CL], F32)
            nc.sync.dma_start(out=sb_invp[:], in_=invc_p[:])

            sb_eps = singles.tile([P, 1], F32)
            nc.vector.memset(sb_eps[:], EPS)
            carry = singles.tile([1, 2], F32)
            nc.vector.memset(carry[:], 0.0)

            gamma_bc = sb_gamma[:].rearrange("p (o h) -> p o h", o=1).to_broadcast(
                (P, G, H)
            )
            if use_beta:
                beta_bc = sb_beta[:].rearrange("p (o h) -> p o h", o=1).to_broadcast(
                    (P, G, H)
                )

            xb = []
            invb = {}
            nmib = {}

            def load_band(b):
                xt = xband.tile([P, CL, H], F32)
                nsub = NGB if b == 0 else 1
                step = CL // nsub
                xv = xr[b]
                bnb = segp.tile([P, CL, 6], F32, tag="bn")
                for u in range(nsub):
                    nc.sync.dma_start(
                        out=xt[:, u * step:(u + 1) * step, :],
                        in_=xv[:, u * step:(u + 1) * step, :],
                    )
                    for r in range(u * step, (u + 1) * step):
                        nc.vector.bn_stats(out=bnb[:, r, :], in_=xt[:, r, :])
                xb.append(xt)
                return bnb

            def scan_band(b, bnb):
                me = bnb[:, :, 1]
                mo = bnb[:, :, 4]
                m2e = bnb[:, :, 2]
                m2o = bnb[:, :, 5]
                # rowsum/128: merge even/odd on the PE via identity-matmul
                # accumulation (I@me + I@mo); likewise rowsumsq picks up
                # m2e + m2o + 128*(me^2 + mo^2) in one PSUM accumulation.
                pe = segp.tile([P, CL], F32, tag="pe")
                nc.vector.scalar_tensor_tensor(
                    out=pe[:], in0=me, scalar=128.0, in1=me,
                    op0=ALU.mult, op1=ALU.mult,
                )
                po = segp.tile([P, CL], F32, tag="po")
                nc.vector.scalar_tensor_tensor(
                    out=po[:], in0=mo, scalar=128.0, in1=mo,
                    op0=ALU.mult, op1=ALU.mult,
                )
                se_ps = psum.tile([P, CL], F32, tag="se_ps")
                nc.tensor.matmul(
                    se_ps[:], lhsT=sb_ident[:], rhs=me, start=True, stop=False
                )
                nc.tensor.matmul(
                    se_ps[:], lhsT=sb_ident[:], rhs=mo, start=False, stop=True
                )
                sp_ps = psum.tile([P, CL], F32, tag="sp_ps")
                nc.tensor.matmul(
                    sp_ps[:], lhsT=sb_ident[:], rhs=m2e, start=True, stop=False
                )
                nc.tensor.matmul(
                    sp_ps[:], lhsT=sb_ident[:], rhs=m2o, start=False, stop=False
                )
                nc.tensor.matmul(
                    sp_ps[:], lhsT=sb_ident[:], rhs=pe[:], start=False, stop=False
                )
                nc.tensor.matmul(
                    sp_ps[:], lhsT=sb_ident[:], rhs=po[:], start=False, stop=True
                )

                # prefix along r within each chunk
                scan_s = segp.tile([P, CL], F32, tag="scan_s")
                nc.vector.tensor_tensor_scan(
                    out=scan_s[:], data0=se_ps[:], data1=pe[:],
                    initial=0.0, op0=ALU.add, op1=ALU.bypass,
                )
                scan_p = segp.tile([P, CL], F32, tag="scan_p")
                nc.vector.tensor_tensor_scan(
                    out=scan_p[:], data0=sp_ps[:], data1=pe[:],
                    initial=0.0, op0=ALU.add, op1=ALU.bypass,
                )

                # chunk totals -> exclusive prefix across partitions (PE)
                tot = segp.tile([P, 2], F32, tag="tot")
                nc.vector.tensor_copy(out=tot[:, 0:1], in_=scan_s[:, CL - 1:CL])
                nc.vector.tensor_copy(out=tot[:, 1:2], in_=scan_p[:, CL - 1:CL])
                offs = psum.tile([P, 2], F32, tag="offs")
                nc.tensor.matmul(
                    offs[:], lhsT=sb_utri[:], rhs=tot[:], start=True, stop=False
                )
                nc.tensor.matmul(
                    offs[:], lhsT=sb_oner[:], rhs=carry[:], start=False, stop=True
                )
                # band total (1,2) for the running carry
                btot = psum.tile([1, 2], F32, tag="btot")
                nc.tensor.matmul(
                    btot[:], lhsT=sb_onec[:], rhs=tot[:], start=True, stop=True
                )
                nc.vector.tensor_add(out=carry[:], in0=carry[:], in1=btot[:])

                # mean / msq / var / rstd / -mean*rstd  (affine-ready layout)
                mean_c = segp.tile([P, CL], F32, tag="mean_c")
                nc.vector.scalar_tensor_tensor(
                    out=mean_c[:], in0=scan_s[:], scalar=offs[:, 0:1],
                    in1=sb_invm[:, b, :], op0=ALU.add, op1=ALU.mult,
                )
                msq_c = segp.tile([P, CL], F32, tag="msq_c")
                nc.vector.scalar_tensor_tensor(
                    out=msq_c[:], in0=scan_p[:], scalar=offs[:, 1:2],
                    in1=sb_invp[:, b, :], op0=ALU.add, op1=ALU.mult,
                )
                var_c = segp.tile([P, CL], F32, tag="var_c")
                nc.vector.tensor_mul(out=var_c[:], in0=mean_c[:], in1=mean_c[:])
                nc.vector.tensor_sub(out=var_c[:], in0=msq_c[:], in1=var_c[:])
                sd_c = segp.tile([P, CL], F32, tag="sd_c")
                nc.scalar.activation(
                    out=sd_c[:], in_=var_c[:], func=ACTF.Sqrt, bias=sb_eps[:],
                )
                inv_c = segp.tile([P, CL], F32, tag="inv_c")
                nc.vector.reciprocal(out=inv_c[:], in_=sd_c[:])
                nmi_c = segp.tile([P, CL], F32, tag="nmi_c")
                nc.vector.scalar_tensor_tensor(
                    out=nmi_c[:], in0=mean_c[:], scalar=-1.0, in1=inv_c[:],
                    op0=ALU.mult, op1=ALU.mult,
                )
                invb[b] = inv_c
                nmib[b] = nmi_c

            obands = {}

            def out_band(b):
                # affine + gamma interleaved per 5-row group so gammas
                # pipeline group-by-group behind the affines
                xt = xb[b]
                inv_c = invb[b]
                nmi_c = nmib[b]
                obs = []
                for j in range(NGB):
                    ob = opool.tile([P, G, H], F32)
                    for jr in range(G):
                        r = j * G + jr
                        if j in AFFINE_GPS_J:
                            nc.gpsimd.tensor_scalar(
                                out=ob[:, jr, :], in0=xt[:, r, :],
                                scalar1=inv_c[:, r:r + 1],
                                scalar2=nmi_c[:, r:r + 1],
                                op0=ALU.mult, op1=ALU.add,
                            )
                        else:
                            nc.scalar.activation(
                                out=ob[:, jr, :], in_=xt[:, r, :],
                                func=ACTF.Identity,
                                bias=nmi_c[:, r:r + 1], scale=inv_c[:, r:r + 1],
                            )
                    geng = nc.vector if j in GAMMA_DVE_J else nc.gpsimd
                    geng.tensor_mul(out=ob[:], in0=ob[:], in1=gamma_bc)
                    if use_beta:
                        geng.tensor_add(out=ob[:], in0=ob[:], in1=beta_bc)
                    obs.append(ob)
                obands[b] = obs

            def store_band(b):
                for j, ob in enumerate(obands[b]):
                    nc.scalar.dma_start(
                        out=yr[b][:, j * G:(j + 1) * G, :], in_=ob[:],
                    )

            # pipelined emission, output lagging one band. Affines come
            # before the next chain in the ACT stream; vector gammas come
            # after the chain in the DVE stream (so the chain never waits
            # behind a gamma that itself waits on ACT affines); store
            # triggers last.
            bn0 = load_band(0)
            scan_band(0, bn0)
            for b in range(1, BANDS):
                bnb = load_band(b)
                out_band(b - 1)
                scan_band(b, bnb)
                store_band(b - 1)
            out_band(BANDS - 1)
            store_band(BANDS - 1)

    nc.compile()
    return nc


_CACHE = {}


def _get(use_beta: bool):
    if use_beta not in _CACHE:
        _CACHE[use_beta] = _build(use_beta)
    return _CACHE[use_beta]


def _make_consts():
    # strictly-upper triangular ones: lhsT[q, p] = 1 iff q < p
    utri = np.triu(np.ones((P, P), dtype=np.float32), k=1)
    ident = np.eye(P, dtype=np.float32)
    ones_col = np.ones((P, 1), dtype=np.float32)
    ones_row = np.ones((1, P), dtype=np.float32)
    k = np.arange(K, dtype=np.float64).reshape(BANDS, P, CL)  # [b, p, r]
    counts = np.transpose(k, (1, 0, 2)) + 1.0                 # [p, b, r]
    invc_m = (1.0 / (2.0 * counts)).astype(np.float32)
    invc_p = (1.0 / (float(H) * counts)).astype(np.float32)
    return utri, ident, ones_col, ones_row, invc_m, invc_p


def _prepare(inputs, gamma, beta):
    inputs = np.ascontiguousarray(inputs, dtype=np.float32)
    gamma = np.asarray(gamma, dtype=np.float32).reshape(1, H)
    beta = np.asarray(beta, dtype=np.float32).reshape(1, H)
    use_beta = bool(np.any(beta))

    gamma_b = np.ascontiguousarray(np.broadcast_to(gamma, (P, H)))
    utri, ident, ones_col, ones_row, invc_m, invc_p = _make_consts()

    in_maps = []
    for b in range(B):
        m = {
            "x": np.ascontiguousarray(inputs[b]),
            "gamma_b": gamma_b,
            "utri": utri,
            "ident": ident,
            "ones_col": ones_col,
            "ones_row": ones_row,
            "invc_m": invc_m,
            "invc_p": invc_p,
        }
        if use_beta:
            m["beta_b"] = np.ascontiguousarray(np.broadcast_to(beta, (P, H)))
        in_maps.append(m)
    return use_beta, in_maps


def kernel(inputs: np.ndarray, gamma: np.ndarray, beta: np.ndarray) -> np.ndarray:
    use_beta, in_maps = _prepare(inputs, gamma, beta)
    nc = _get(use_beta)
    res = run_bass_kernel_spmd(nc, in_maps, list(range(B)))
    out = np.stack([res.results[b]["y"] for b in range(B)], axis=0)
    return out


# revision 14
# speedup vs baseline: 1.2871x; 1.0351x over previous
"""Causal (cumulative) LayerNorm Trainium2 Bass kernel.

Full-input contract: kernel(inputs, gamma, beta) takes the full
(B=8, K=16000, H=256) f32 tensor, shards batch across 8 NeuronCores
(one sample per core), and returns the full (8, 16000, 256) output.

Per-core algorithm (x is (K, H)):
  rowsum[k]   = sum_h x[k, h]
  rowsumsq[k] = sum_h x[k, h]^2
  csum = cumsum(rowsum); cpow = cumsum(rowsumsq)
  mean[k] = csum[k] / (H*(k+1));  msq[k] = cpow[k] / (H*(k+1))
  var[k] = msq[k] - mean[k]^2
  out[k, h] = gamma[h] * (x[k, h] - mean[k]) / sqrt(var[k] + EPS) + beta[h]

Layout: row k = b*3200 + p*25 + r for band b in 0..4, partition p in
0..127, r in 0..24. Each band is one (128, 25, 256) SBUF tile whose
per-partition 25 rows are CONTIGUOUS in HBM, so band loads/stores are
single DMA triggers with 25 KB contiguous runs per partition (max DMA
efficiency, negligible descriptor-gen on the sequencers).

Per band: one bn_stats per r (even/odd mean/M2 per row), six cheap
full-partition merges to rowsum/128 and rowsumsq, a fp32 vector scan
along r (prefix within each 25-row chunk), chunk totals prefix-summed
ACROSS partitions by one strictly-triangular PE matmul (PE fp32 is
full-precision, ~6e-8 verified) plus a broadcast matmul seeding the
running inter-band carry, then per-row rstd / -mean*rstd computed
directly in affine-ready layout (no transposes anywhere). Output pass
runs IN PLACE over the band tile: per-r affine (scalar engine mostly),
batched gamma multiply (gpsimd/vector), one store trigger per r-group.
Bands pipeline: band b's output overlaps band b+1's load. DMA is the
roofline: ~32.8 MB @ ~358 GB/s/core.
"""

import numpy as np

import concourse.bass as bass
import concourse.bacc as bacc
import concourse.tile as tile
from concourse import mybir
from concourse.bass_utils import run_bass_kernel_spmd

EPS = 1e-8
B, K, H = 8, 16000, 256
P = 128                  # SBUF partitions = chunks per band
CL = 25                  # rows per chunk (per partition per band)
BANDS = K // (P * CL)    # 5
G = 5                    # rows per gamma/store group
NGB = CL // G            # 5 groups per band
F32 = mybir.dt.float32
ALU = mybir.AluOpType
ACTF = mybir.ActivationFunctionType

GAMMA_DVE_J = {2}        # per-band group index -> gamma on vector, rest gpsimd
AFFINE_GPS_J = {1, 3}    # per-band group index -> affine on gpsimd, rest scalar


def _build(use_beta: bool):
    nc = bacc.Bacc("TRN2", target_bir_lowering=False, debug=False)

    x = nc.declare_dram_parameter("x", [K, H], F32, isOutput=False)
    gamma_b = nc.declare_dram_parameter("gamma_b", [P, H], F32, isOutput=False)
    beta_b = (
        nc.declare_dram_parameter("beta_b", [P, H], F32, isOutput=False)
        if use_beta
        else None
    )
    utri = nc.declare_dram_parameter("utri", [P, P], F32, isOutput=False)
    ident = nc.declare_dram_parameter("ident", [P, P], F32, isOutput=False)
    ones_col = nc.declare_dram_parameter("ones_col", [P, 1], F32, isOutput=False)
    ones_row = nc.declare_dram_parameter("ones_row", [1, P], F32, isOutput=False)
    invc_m = nc.declare_dram_parameter("invc_m", [P, BANDS, CL], F32, isOutput=False)
    invc_p = nc.declare_dram_parameter("invc_p", [P, BANDS, CL], F32, isOutput=False)
    y = nc.declare_dram_parameter("y", [K, H], F32, isOutput=True)

    xr = x.rearrange("(b p r) h -> b p r h", p=P, r=CL)   # [5, 128, 25, 256]
    yr = y.rearrange("(b p r) h -> b p r h", p=P, r=CL)

    with tile.TileContext(nc) as tc:
        with (
            tc.tile_pool(name="singles", bufs=1) as singles,
            tc.tile_pool(name="xband", bufs=BANDS) as xband,
            tc.tile_pool(name="opool", bufs=10) as opool,
            tc.tile_pool(name="segp", bufs=3) as segp,
            tc.tile_pool(name="psum", bufs=2, space="PSUM") as psum,
        ):
            sb_gamma = singles.tile([P, H], F32)
            nc.sync.dma_start(out=sb_gamma[:], in_=gamma_b[:])
            if use_beta:
                sb_beta = singles.tile([P, H], F32)
                nc.sync.dma_start(out=sb_beta[:], in_=beta_b[:])
            sb_utri = singles.tile([P, P], F32)
            nc.sync.dma_start(out=sb_utri[:], in_=utri[:])
            sb_ident = singles.tile([P, P], F32)
            nc.sync.dma_start(out=sb_ident[:], in_=ident[:])
            sb_onec = singles.tile([P, 1], F32)
            nc.sync.dma_start(out=sb_onec[:], in_=ones_col[:])
            sb_oner = singles.tile([1, P], F32)
            nc.sync.dma_start(out=sb_oner[:], in_=ones_row[:])
            sb_invm = singles.tile([P, BANDS, CL], F32)
            nc.sync.dma_start(out=sb_invm[:], in_=invc_m[:])
            sb_invp = singles.tile([P, BANDS, CL], F32)
            nc.sync.dma_start(out=sb_invp[:], in_=invc_p[:])

            sb_eps = singles.tile([P, 1], F32)
            nc.vector.memset(sb_eps[:], EPS)
            carry = singles.tile([1, 2], F32)
            nc.vector.memset(carry[:], 0.0)

            gamma_bc = sb_gamma[:].rearrange("p (o h) -> p o h", o=1).to_broadcast(
                (P, G, H)
            )
            if use_beta:
                beta_bc = sb_beta[:].rearrange("p (o h) -> p o h", o=1).to_broadcast(
                    (P, G, H)
                )

            xb = []
            invb = {}
            nmib = {}

            def load_band(b):
                xt = xband.tile([P, CL, H], F32)
                nsub = NGB if b == 0 else 1
                step = CL // nsub
                xv = xr[b]
                bnb = segp.tile([P, CL, 6], F32, tag="bn")
                for u in range(nsub):
                    nc.sync.dma_start(
                        out=xt[:, u * step:(u + 1) * step, :],
                        in_=xv[:, u * step:(u + 1) * step, :],
                    )
                    for r in range(u * step, (u + 1) * step):
                        nc.vector.bn_stats(out=bnb[:, r, :], in_=xt[:, r, :])
                xb.append(xt)
                return bnb

            def scan_band(b, bnb):
                me = bnb[:, :, 1]
                mo = bnb[:, :, 4]
                m2e = bnb[:, :, 2]
                m2o = bnb[:, :, 5]
                # rowsum/128: merge even/odd on the PE via identity-matmul
                # accumulation (I@me + I@mo); likewise rowsumsq picks up
                # m2e + m2o + 128*(me^2 + mo^2) in one PSUM accumulation.
                pe = segp.tile([P, CL], F32, tag="pe")
                nc.vector.scalar_tensor_tensor(
                    out=pe[:], in0=me, scalar=128.0, in1=me,
                    op0=ALU.mult, op1=ALU.mult,
                )
                po = segp.tile([P, CL], F32, tag="po")
                nc.vector.scalar_tensor_tensor(
                    out=po[:], in0=mo, scalar=128.0, in1=mo,
                    op0=ALU.mult, op1=ALU.mult,
                )
                se_ps = psum.tile([P, CL], F32, tag="se_ps")
                nc.tensor.matmul(
                    se_ps[:], lhsT=sb_ident[:], rhs=me, start=True, stop=False
                )
                nc.tensor.matmul(
                    se_ps[:], lhsT=sb_ident[:], rhs=mo, start=False, stop=True
                )
                sp_ps = psum.tile([P, CL], F32, tag="sp_ps")
                nc.tensor.matmul(
                    sp_ps[:], lhsT=sb_ident[:], rhs=m2e, start=True, stop=False
                )
                nc.tensor.matmul(
                    sp_ps[:], lhsT=sb_ident[:], rhs=m2o, start=False, stop=False
                )
                nc.tensor.matmul(
                    sp_ps[:], lhsT=sb_ident[:], rhs=pe[:], start=False, stop=False
                )
                nc.tensor.matmul(
                    sp_ps[:], lhsT=sb_ident[:], rhs=po[:], start=False, stop=True
                )

                # prefix along r within each chunk
                scan_s = segp.tile([P, CL], F32, tag="scan_s")
                nc.vector.tensor_tensor_scan(
                    out=scan_s[:], data0=se_ps[:], data1=pe[:],
                    initial=0.0, op0=ALU.add, op1=ALU.bypass,
                )
                scan_p = segp.tile([P, CL], F32, tag="scan_p")
                nc.vector.tensor_tensor_scan(
                    out=scan_p[:], data0=sp_ps[:], data1=pe[:],
                    initial=0.0, op0=ALU.add, op1=ALU.bypass,
                )

                # chunk totals -> exclusive prefix across partitions (PE)
                tot = segp.tile([P, 2], F32, tag="tot")
                nc.vector.tensor_copy(out=tot[:, 0:1], in_=scan_s[:, CL - 1:CL])
                nc.vector.tensor_copy(out=tot[:, 1:2], in_=scan_p[:, CL - 1:CL])
                offs = psum.tile([P, 2], F32, tag="offs")
                nc.tensor.matmul(
                    offs[:], lhsT=sb_utri[:], rhs=tot[:], start=True, stop=False
                )
                nc.tensor.matmul(
                    offs[:], lhsT=sb_oner[:], rhs=carry[:], start=False, stop=True
                )
                # band total (1,2) for the running carry
                btot = psum.tile([1, 2], F32, tag="btot")
                nc.tensor.matmul(
                    btot[:], lhsT=sb_onec[:], rhs=tot[:], start=True, stop=True
                )
                nc.vector.tensor_add(out=carry[:], in0=carry[:], in1=btot[:])

                # mean / msq / var / rstd / -mean*rstd  (affine-ready layout)
                mean_c = segp.tile([P, CL], F32, tag="mean_c")
                nc.vector.scalar_tensor_tensor(
                    out=mean_c[:], in0=scan_s[:], scalar=offs[:, 0:1],
                    in1=sb_invm[:, b, :], op0=ALU.add, op1=ALU.mult,
                )
                msq_c = segp.tile([P, CL], F32, tag="msq_c")
                nc.vector.scalar_tensor_tensor(
                    out=msq_c[:], in0=scan_p[:], scalar=offs[:, 1:2],
                    in1=sb_invp[:, b, :], op0=ALU.add, op1=ALU.mult,
                )
                var_c = segp.tile([P, CL], F32, tag="var_c")
                nc.vector.tensor_mul(out=var_c[:], in0=mean_c[:], in1=mean_c[:])
                nc.vector.tensor_sub(out=var_c[:], in0=msq_c[:], in1=var_c[:])
                sd_c = segp.tile([P, CL], F32, tag="sd_c")
                nc.scalar.activation(
                    out=sd_c[:], in_=var_c[:], func=ACTF.Sqrt, bias=sb_eps[:],
                )
                inv_c = segp.tile([P, CL], F32, tag="inv_c")
                nc.vector.reciprocal(out=inv_c[:], in_=sd_c[:])
                nmi_c = segp.tile([P, CL], F32, tag="nmi_c")
                nc.vector.scalar_tensor_tensor(
                    out=nmi_c[:], in0=mean_c[:], scalar=-1.0, in1=inv_c[:],
                    op0=ALU.mult, op1=ALU.mult,
                )
                invb[b] = inv_c
                nmib[b] = nmi_c

            obands = {}

            def out_band(b):
                # affine + gamma interleaved per 5-row group so gammas
                # pipeline group-by-group behind the affines
                xt = xb[b]
                inv_c = invb[b]
                nmi_c = nmib[b]
                obs = []
                for j in range(NGB):
                    ob = opool.tile([P, G, H], F32)
                    for jr in range(G):
                        r = j * G + jr
                        if j in AFFINE_GPS_J:
                            nc.gpsimd.tensor_scalar(
                                out=ob[:, jr, :], in0=xt[:, r, :],
                                scalar1=inv_c[:, r:r + 1],
                                scalar2=nmi_c[:, r:r + 1],
                                op0=ALU.mult, op1=ALU.add,
                            )
                        else:
                            nc.scalar.activation(
                                out=ob[:, jr, :], in_=xt[:, r, :],
                                func=ACTF.Identity,
                                bias=nmi_c[:, r:r + 1], scale=inv_c[:, r:r + 1],
                            )
                    geng = nc.vector if j in GAMMA_DVE_J else nc.gpsimd
                    geng.tensor_mul(out=ob[:], in0=ob[:], in1=gamma_bc)
                    if use_beta:
                        geng.tensor_add(out=ob[:], in0=ob[:], in1=beta_bc)
                    obs.append(ob)
                obands[b] = obs

            def store_band(b):
                for j, ob in enumerate(obands[b]):
                    nc.scalar.dma_start(
                        out=yr[b][:, j * G:(j + 1) * G, :], in_=ob[:],
                    )

            # pipelined emission, output lagging one band. Affines come
            # before the next chain in the ACT stream; vector gammas come
            # after the chain in the DVE stream (so the chain never waits
            # behind a gamma that itself waits on ACT affines); store
            # triggers last.
            bn0 = load_band(0)
            scan_band(0, bn0)
            for b in range(1, BANDS):
                bnb = load_band(b)
                out_band(b - 1)
                scan_band(b, bnb)
                store_band(b - 1)
            out_band(BANDS - 1)
            store_band(BANDS - 1)

    nc.compile()
    return nc


_CACHE = {}


def _get(use_beta: bool):
    if use_beta not in _CACHE:
        _CACHE[use_beta] = _build(use_beta)
    return _CACHE[use_beta]


def _make_consts():
    # strictly-upper triangular ones: lhsT[q, p] = 1 iff q < p
    utri = np.triu(np.ones((P, P), dtype=np.float32), k=1)
    ident = np.eye(P, dtype=np.float32)
    ones_col = np.ones((P, 1), dtype=np.float32)
    ones_row = np.ones((1, P), dtype=np.float32)
    k = np.arange(K, dtype=np.float64).reshape(BANDS, P, CL)  # [b, p, r]
    counts = np.transpose(k, (1, 0, 2)) + 1.0                 # [p, b, r]
    invc_m = (1.0 / (2.0 * counts)).astype(np.float32)
    invc_p = (1.0 / (float(H) * counts)).astype(np.float32)
    return utri, ident, ones_col, ones_row, invc_m, invc_p


def _prepare(inputs, gamma, beta):
    inputs = np.ascontiguousarray(inputs, dtype=np.float32)
    gamma = np.asarray(gamma, dtype=np.float32).reshape(1, H)
    beta = np.asarray(beta, dtype=np.float32).reshape(1, H)
    use_beta = bool(np.any(beta))

    gamma_b = np.ascontiguousarray(np.broadcast_to(gamma, (P, H)))
    utri, ident, ones_col, ones_row, invc_m, invc_p = _make_consts()

    in_maps = []
    for b in range(B):
        m = {
            "x": np.ascontiguousarray(inputs[b]),
            "gamma_b": gamma_b,
            "utri": utri,
            "ident": ident,
            "ones_col": ones_col,
            "ones_row": ones_row,
            "invc_m": invc_m,
            "invc_p": invc_p,
        }
        if use_beta:
            m["beta_b"] = np.ascontiguousarray(np.broadcast_to(beta, (P, H)))
        in_maps.append(m)
    return use_beta, in_maps


def kernel(inputs: np.ndarray, gamma: np.ndarray, beta: np.ndarray) -> np.ndarray:
    use_beta, in_maps = _prepare(inputs, gamma, beta)
    nc = _get(use_beta)
    res = run_bass_kernel_spmd(nc, in_maps, list(range(B)))
    out = np.stack([res.results[b]["y"] for b in range(B)], axis=0)
    return out


# revision 15
# speedup vs baseline: 1.2902x; 1.0024x over previous
"""Causal (cumulative) LayerNorm Trainium2 Bass kernel.

Full-input contract: kernel(inputs, gamma, beta) takes the full
(B=8, K=16000, H=256) f32 tensor, shards batch across 8 NeuronCores
(one sample per core), and returns the full (8, 16000, 256) output.

Per-core algorithm (x is (K, H)):
  rowsum[k]   = sum_h x[k, h]
  rowsumsq[k] = sum_h x[k, h]^2
  csum = cumsum(rowsum); cpow = cumsum(rowsumsq)
  mean[k] = csum[k] / (H*(k+1));  msq[k] = cpow[k] / (H*(k+1))
  var[k] = msq[k] - mean[k]^2
  out[k, h] = gamma[h] * (x[k, h] - mean[k]) / sqrt(var[k] + EPS) + beta[h]

Layout: row k = b*3200 + p*25 + r for band b in 0..4, partition p in
0..127, r in 0..24. Each band is one (128, 25, 256) SBUF tile whose
per-partition 25 rows are CONTIGUOUS in HBM, so band loads/stores are
single DMA triggers with 25 KB contiguous runs per partition (max DMA
efficiency, negligible descriptor-gen on the sequencers).

Per band: one bn_stats per r (even/odd mean/M2 per row), six cheap
full-partition merges to rowsum/128 and rowsumsq, a fp32 vector scan
along r (prefix within each 25-row chunk), chunk totals prefix-summed
ACROSS partitions by one strictly-triangular PE matmul (PE fp32 is
full-precision, ~6e-8 verified) plus a broadcast matmul seeding the
running inter-band carry, then per-row rstd / -mean*rstd computed
directly in affine-ready layout (no transposes anywhere). Output pass
runs IN PLACE over the band tile: per-r affine (scalar engine mostly),
batched gamma multiply (gpsimd/vector), one store trigger per r-group.
Bands pipeline: band b's output overlaps band b+1's load. DMA is the
roofline: ~32.8 MB @ ~358 GB/s/core.
"""

import numpy as np

import concourse.bass as bass
import concourse.bacc as bacc
import concourse.tile as tile
from concourse import mybir
from concourse.bass_utils import run_bass_kernel_spmd

EPS = 1e-8
B, K, H = 8, 16000, 256
P = 128                  # SBUF partitions = chunks per band
CL = 25                  # rows per chunk (per partition per band)
BANDS = K // (P * CL)    # 5
G = 5                    # rows per gamma/store group
NGB = CL // G            # 5 groups per band
F32 = mybir.dt.float32
ALU = mybir.AluOpType
ACTF = mybir.ActivationFunctionType

GAMMA_DVE_J = {2}        # per-band group index -> gamma on vector, rest gpsimd
AFFINE_GPS_J = {1, 3}    # per-band group index -> affine on gpsimd, rest scalar


def _build(use_beta: bool):
    nc = bacc.Bacc("TRN2", target_bir_lowering=False, debug=False)

    x = nc.declare_dram_parameter("x", [K, H], F32, isOutput=False)
    gamma_b = nc.declare_dram_parameter("gamma_b", [P, H], F32, isOutput=False)
    beta_b = (
        nc.declare_dram_parameter("beta_b", [P, H], F32, isOutput=False)
        if use_beta
        else None
    )
    utri = nc.declare_dram_parameter("utri", [P, P], F32, isOutput=False)
    ident = nc.declare_dram_parameter("ident", [P, P], F32, isOutput=False)
    ones_col = nc.declare_dram_parameter("ones_col", [P, 1], F32, isOutput=False)
    ones_row = nc.declare_dram_parameter("ones_row", [1, P], F32, isOutput=False)
    invc_m = nc.declare_dram_parameter("invc_m", [P, BANDS, CL], F32, isOutput=False)
    invc_p = nc.declare_dram_parameter("invc_p", [P, BANDS, CL], F32, isOutput=False)
    y = nc.declare_dram_parameter("y", [K, H], F32, isOutput=True)

    xr = x.rearrange("(b p r) h -> b p r h", p=P, r=CL)   # [5, 128, 25, 256]
    yr = y.rearrange("(b p r) h -> b p r h", p=P, r=CL)

    with tile.TileContext(nc) as tc:
        with (
            tc.tile_pool(name="singles", bufs=1) as singles,
            tc.tile_pool(name="xband", bufs=BANDS) as xband,
            tc.tile_pool(name="opool", bufs=10) as opool,
            tc.tile_pool(name="segp", bufs=3) as segp,
            tc.tile_pool(name="psum", bufs=2, space="PSUM") as psum,
        ):
            sb_gamma = singles.tile([P, H], F32)
            nc.sync.dma_start(out=sb_gamma[:], in_=gamma_b[:])
            if use_beta:
                sb_beta = singles.tile([P, H], F32)
                nc.sync.dma_start(out=sb_beta[:], in_=beta_b[:])
            sb_utri = singles.tile([P, P], F32)
            nc.sync.dma_start(out=sb_utri[:], in_=utri[:])
            sb_ident = singles.tile([P, P], F32)
            nc.sync.dma_start(out=sb_ident[:], in_=ident[:])
            sb_onec = singles.tile([P, 1], F32)
            nc.sync.dma_start(out=sb_onec[:], in_=ones_col[:])
            sb_oner = singles.tile([1, P], F32)
            nc.sync.dma_start(out=sb_oner[:], in_=ones_row[:])
            sb_invm = singles.tile([P, BANDS, CL], F32)
            nc.sync.dma_start(out=sb_invm[:], in_=invc_m[:])
            sb_invp = singles.tile([P, BANDS, CL], F32)
            nc.sync.dma_start(out=sb_invp[:], in_=invc_p[:])

            sb_eps = singles.tile([P, 1], F32)
            nc.vector.memset(sb_eps[:], EPS)
            carry = singles.tile([1, 2], F32)
            nc.vector.memset(carry[:], 0.0)

            gamma_bc = sb_gamma[:].rearrange("p (o h) -> p o h", o=1).to_broadcast(
                (P, G, H)
            )
            if use_beta:
                beta_bc = sb_beta[:].rearrange("p (o h) -> p o h", o=1).to_broadcast(
                    (P, G, H)
                )

            xb = []
            invb = {}
            nmib = {}

            def load_band(b):
                xt = xband.tile([P, CL, H], F32)
                nsub = NGB if b == 0 else 1
                step = CL // nsub
                xv = xr[b]
                bnb = segp.tile([P, CL, 6], F32, tag="bn")
                for u in range(nsub):
                    nc.sync.dma_start(
                        out=xt[:, u * step:(u + 1) * step, :],
                        in_=xv[:, u * step:(u + 1) * step, :],
                    )
                    for r in range(u * step, (u + 1) * step):
                        nc.vector.bn_stats(out=bnb[:, r, :], in_=xt[:, r, :])
                xb.append(xt)
                return bnb

            def scan_band(b, bnb):
                me = bnb[:, :, 1]
                mo = bnb[:, :, 4]
                m2e = bnb[:, :, 2]
                m2o = bnb[:, :, 5]
                # rowsum/128: merge even/odd on the PE via identity-matmul
                # accumulation (I@me + I@mo); likewise rowsumsq picks up
                # m2e + m2o + 128*(me^2 + mo^2) in one PSUM accumulation.
                pe = segp.tile([P, CL], F32, tag="pe")
                nc.vector.scalar_tensor_tensor(
                    out=pe[:], in0=me, scalar=128.0, in1=me,
                    op0=ALU.mult, op1=ALU.mult,
                )
                po = segp.tile([P, CL], F32, tag="po")
                nc.vector.scalar_tensor_tensor(
                    out=po[:], in0=mo, scalar=128.0, in1=mo,
                    op0=ALU.mult, op1=ALU.mult,
                )
                se_ps = psum.tile([P, CL], F32, tag="se_ps")
                nc.tensor.matmul(
                    se_ps[:], lhsT=sb_ident[:], rhs=me, start=True, stop=False
                )
                nc.tensor.matmul(
                    se_ps[:], lhsT=sb_ident[:], rhs=mo, start=False, stop=True
                )
                sp_ps = psum.tile([P, CL], F32, tag="sp_ps")
                nc.tensor.matmul(
                    sp_ps[:], lhsT=sb_ident[:], rhs=m2e, start=True, stop=False
                )
                nc.tensor.matmul(
                    sp_ps[:], lhsT=sb_ident[:], rhs=m2o, start=False, stop=False
                )
                nc.tensor.matmul(
                    sp_ps[:], lhsT=sb_ident[:], rhs=pe[:], start=False, stop=False
                )
                nc.tensor.matmul(
                    sp_ps[:], lhsT=sb_ident[:], rhs=po[:], start=False, stop=True
                )

                # prefix along r within each chunk
                scan_s = segp.tile([P, CL], F32, tag="scan_s")
                nc.vector.tensor_tensor_scan(
                    out=scan_s[:], data0=se_ps[:], data1=pe[:],
                    initial=0.0, op0=ALU.add, op1=ALU.bypass,
                )
                scan_p = segp.tile([P, CL], F32, tag="scan_p")
                nc.vector.tensor_tensor_scan(
                    out=scan_p[:], data0=sp_ps[:], data1=pe[:],
                    initial=0.0, op0=ALU.add, op1=ALU.bypass,
                )

                # chunk totals -> exclusive prefix across partitions (PE)
                tot = segp.tile([P, 2], F32, tag="tot")
                nc.vector.tensor_copy(out=tot[:, 0:1], in_=scan_s[:, CL - 1:CL])
                nc.vector.tensor_copy(out=tot[:, 1:2], in_=scan_p[:, CL - 1:CL])
                offs = psum.tile([P, 2], F32, tag="offs")
                nc.tensor.matmul(
                    offs[:], lhsT=sb_utri[:], rhs=tot[:], start=True, stop=False
                )
                nc.tensor.matmul(
                    offs[:], lhsT=sb_oner[:], rhs=carry[:], start=False, stop=True
                )
                # band total (1,2) for the running carry
                btot = psum.tile([1, 2], F32, tag="btot")
                nc.tensor.matmul(
                    btot[:], lhsT=sb_onec[:], rhs=tot[:], start=True, stop=True
                )
                nc.vector.tensor_add(out=carry[:], in0=carry[:], in1=btot[:])

                # mean / msq / var / rstd / -mean*rstd  (affine-ready layout)
                mean_c = segp.tile([P, CL], F32, tag="mean_c")
                nc.vector.scalar_tensor_tensor(
                    out=mean_c[:], in0=scan_s[:], scalar=offs[:, 0:1],
                    in1=sb_invm[:, b, :], op0=ALU.add, op1=ALU.mult,
                )
                msq_c = segp.tile([P, CL], F32, tag="msq_c")
                nc.vector.scalar_tensor_tensor(
                    out=msq_c[:], in0=scan_p[:], scalar=offs[:, 1:2],
                    in1=sb_invp[:, b, :], op0=ALU.add, op1=ALU.mult,
                )
                var_c = segp.tile([P, CL], F32, tag="var_c")
                nc.vector.tensor_mul(out=var_c[:], in0=mean_c[:], in1=mean_c[:])
                nc.vector.tensor_sub(out=var_c[:], in0=msq_c[:], in1=var_c[:])
                sd_c = segp.tile([P, CL], F32, tag="sd_c")
                nc.scalar.activation(
                    out=sd_c[:], in_=var_c[:], func=ACTF.Sqrt, bias=sb_eps[:],
                )
                inv_c = segp.tile([P, CL], F32, tag="inv_c")
                nc.vector.reciprocal(out=inv_c[:], in_=sd_c[:])
                nmi_c = segp.tile([P, CL], F32, tag="nmi_c")
                nc.vector.scalar_tensor_tensor(
                    out=nmi_c[:], in0=mean_c[:], scalar=-1.0, in1=inv_c[:],
                    op0=ALU.mult, op1=ALU.mult,
                )
                invb[b] = inv_c
                nmib[b] = nmi_c

            obands = {}

            # last band runs after all loads: vector is idle then, so its
            # output pass spreads across all three engines
            LAST_AFF = ["act", "gps", "dve", "act", "dve"]
            LAST_GAM = ["dve", "gps", "dve", "gps", "dve"]

            def out_band(b):
                # affine + gamma interleaved per 5-row group so gammas
                # pipeline group-by-group behind the affines
                xt = xb[b]
                inv_c = invb[b]
                nmi_c = nmib[b]
                last = b == BANDS - 1
                obs = []
                for j in range(NGB):
                    ob = opool.tile([P, G, H], F32)
                    if last:
                        aeng = LAST_AFF[j]
                    else:
                        aeng = "gps" if j in AFFINE_GPS_J else "act"
                    for jr in range(G):
                        r = j * G + jr
                        if aeng == "act":
                            nc.scalar.activation(
                                out=ob[:, jr, :], in_=xt[:, r, :],
                                func=ACTF.Identity,
                                bias=nmi_c[:, r:r + 1], scale=inv_c[:, r:r + 1],
                            )
                        else:
                            eng = nc.gpsimd if aeng == "gps" else nc.vector
                            eng.tensor_scalar(
                                out=ob[:, jr, :], in0=xt[:, r, :],
                                scalar1=inv_c[:, r:r + 1],
                                scalar2=nmi_c[:, r:r + 1],
                                op0=ALU.mult, op1=ALU.add,
                            )
                    if last:
                        geng = nc.vector if LAST_GAM[j] == "dve" else nc.gpsimd
                    else:
                        geng = nc.vector if j in GAMMA_DVE_J else nc.gpsimd
                    geng.tensor_mul(out=ob[:], in0=ob[:], in1=gamma_bc)
                    if use_beta:
                        geng.tensor_add(out=ob[:], in0=ob[:], in1=beta_bc)
                    obs.append(ob)
                obands[b] = obs

            def store_band(b):
                for j, ob in enumerate(obands[b]):
                    nc.scalar.dma_start(
                        out=yr[b][:, j * G:(j + 1) * G, :], in_=ob[:],
                    )

            # pipelined emission, output lagging one band. Affines come
            # before the next chain in the ACT stream; vector gammas come
            # after the chain in the DVE stream (so the chain never waits
            # behind a gamma that itself waits on ACT affines); store
            # triggers last.
            bn0 = load_band(0)
            scan_band(0, bn0)
            for b in range(1, BANDS):
                bnb = load_band(b)
                out_band(b - 1)
                scan_band(b, bnb)
                store_band(b - 1)
            out_band(BANDS - 1)
            store_band(BANDS - 1)

    nc.compile()
    return nc


_CACHE = {}


def _get(use_beta: bool):
    if use_beta not in _CACHE:
        _CACHE[use_beta] = _build(use_beta)
    return _CACHE[use_beta]


def _make_consts():
    # strictly-upper triangular ones: lhsT[q, p] = 1 iff q < p
    utri = np.triu(np.ones((P, P), dtype=np.float32), k=1)
    ident = np.eye(P, dtype=np.float32)
    ones_col = np.ones((P, 1), dtype=np.float32)
    ones_row = np.ones((1, P), dtype=np.float32)
    k = np.arange(K, dtype=np.float64).reshape(BANDS, P, CL)  # [b, p, r]
    counts = np.transpose(k, (1, 0, 2)) + 1.0                 # [p, b, r]
    invc_m = (1.0 / (2.0 * counts)).astype(np.float32)
    invc_p = (1.0 / (float(H) * counts)).astype(np.float32)
    return utri, ident, ones_col, ones_row, invc_m, invc_p


def _prepare(inputs, gamma, beta):
    inputs = np.ascontiguousarray(inputs, dtype=np.float32)
    gamma = np.asarray(gamma, dtype=np.float32).reshape(1, H)
    beta = np.asarray(beta, dtype=np.float32).reshape(1, H)
    use_beta = bool(np.any(beta))

    gamma_b = np.ascontiguousarray(np.broadcast_to(gamma, (P, H)))
    utri, ident, ones_col, ones_row, invc_m, invc_p = _make_consts()

    in_maps = []
    for b in range(B):
        m = {
            "x": np.ascontiguousarray(inputs[b]),
            "gamma_b": gamma_b,
            "utri": utri,
            "ident": ident,
            "ones_col": ones_col,
            "ones_row": ones_row,
            "invc_m": invc_m,
            "invc_p": invc_p,
        }
        if use_beta:
            m["beta_b"] = np.ascontiguousarray(np.broadcast_to(beta, (P, H)))
        in_maps.append(m)
    return use_beta, in_maps


def kernel(inputs: np.ndarray, gamma: np.ndarray, beta: np.ndarray) -> np.ndarray:
    use_beta, in_maps = _prepare(inputs, gamma, beta)
    nc = _get(use_beta)
    res = run_bass_kernel_spmd(nc, in_maps, list(range(B)))
    out = np.stack([res.results[b]["y"] for b in range(B)], axis=0)
    return out
